# revision 31
# baseline (speedup 1.0000x reference)
"""Trainium2 Bass kernel for nn_DConv (shift-gather + 3x3 conv), 8 NeuronCores.

Math: the reference's per-channel torch.roll on the zero-padded image only
ever wraps in zero-pad rows/columns, so the whole op collapses to

    out[b,co,h,w] = sum_{ci,kh,kw} W[co,ci,kh,kw] * x[b,ci, h+kh-1-dy[ci], w+kw-1-dx[ci]]

with out-of-range x treated as 0 and (dy,dx) the c%5 shift table.  On device
we materialise a per-channel shifted + padded copy Xs[ci, u, v] (u,v in
[0,162)) in SBUF via DMA offsets, then run the 3x3 conv as 9 accumulating
PE matmuls over flat windows of Xs (input and output both at row pitch 162,
so each tap is a constant flat offset).

Sharding: data-parallel over batch, 2 samples per core.  SBUF partitions
hold both samples' channels grouped by shift group and interleaved
(g0s0|g0s1|g1s0|...), so each shift group loads both samples in one DMA;
each matmul uses a sample-block-masked [128,128] stationary weight so one
instruction computes the tap for both samples (K=128, M=128 -> full PE
array; out partitions 0-63 = sample 0, 64-127 = sample 1).

Dtype: float32r (fp32 with 11-bit mantissa, PE streams 1 col/cycle at
N>=256) with fp32 PSUM accumulation.  Inputs are pre-rounded to f32r on the
host so host and HW agree bit-exactly.
"""
import numpy as np

from concourse import bacc, tile, mybir
from concourse.bass_utils import run_bass_kernel_spmd

# problem shape (hardcoded per contract)
B, C, H, W = 16, 64, 160, 160
N_CORES = 8
B_PER_CORE = B // N_CORES  # 2
VP = H + 2  # padded pitch 162

# shift table: group g = ci % 5
DXS = [0, 1, 0, -1, 0]
DYS = [0, 0, 1, 0, -1]
# partition layout: for each group g (channels ci%5==g), a block of
# 2*gs partitions: sample0's gs channels then sample1's
GROUP_SIZES = [13, 13, 13, 13, 12]
GROUP_P0 = [0, 26, 52, 78, 104]   # 2 * cumulative offsets

# partition p -> (sample, channel) map, shared by host packing and weights
PART_SAMPLE = np.zeros(128, np.int64)
PART_CHANNEL = np.zeros(128, np.int64)
for _g in range(5):
    _gs = GROUP_SIZES[_g]
    for _sm in range(2):
        for _j in range(_gs):
            _p = GROUP_P0[_g] + _sm * _gs + _j
            PART_SAMPLE[_p] = _sm
            PART_CHANNEL[_p] = 5 * _j + _g

# tiling: variable strips — a small first strip shortens the initial load
# the PE has to wait for; the last strip stores in finer blocks so the
# kernel tail is short.  Each strip entry is (h0, rows, chunk_plan,
# store_bounds): chunk_plan gives output rows per PSUM chunk (each
# N = rows*162 must be in [256, 512] so f32r streams at 1 col/cycle and
# fits one PSUM bank); store_bounds are chunk-aligned row offsets where
# store DMAs fire.
_STD_PLAN = [3, 3, 3, 3, 3, 3, 2] * 2
_LAST_PLAN = [3, 3, 3, 3, 3, 3, 2] + [2, 2, 3, 3] + [3, 3, 2, 2]
STRIP_LIST = [
    (0, 40, _STD_PLAN, (20, 40)),
    (40, 40, _STD_PLAN, (20, 40)),
    (80, 40, _STD_PLAN, (20, 40)),
    (120, 40, _LAST_PLAN, (20, 30, 40)),
]
MAX_STRIP = max(r for _, r, _, _ in STRIP_LIST)     # 48
MIN_STRIP = min(r for _, r, _, _ in STRIP_LIST)     # 24
XS_ROWS = MAX_STRIP + 3    # strip buffer rows (+1 halo each side +1 guard)

XS_BUFS = 3
STG_BUFS = 2
PSUM_BUFS = 4
WARMUP_MMS = 16            # dummy matmuls to ramp the PE during the first load
COPY_ENGINES = ("vector",)  # round-robin engines for PSUM->SBUF copies

TAPS = [(kh, kw) for kh in range(3) for kw in range(3)]


def build_kernel(reps: int = 1, timing: bool = False):
    nc = bacc.Bacc("TRN2", target_bir_lowering=False, debug=False,
                   num_devices=N_CORES)
    wt_dram = nc.dram_tensor("wt", [128, 9, 128], mybir.dt.float32r,
                             kind="ExternalInput")
    if timing:
        # timing-only variant: big tensors stay in device DRAM (uninitialised
        # garbage is fine for timing) so per-call host<->device transfer is
        # tiny and wall-clock noise is dominated by the fixed RTT only.
        x_dram = nc.dram_tensor("x", [2 * C, H, W], mybir.dt.float32r)
        out_dram = nc.dram_tensor("out", [B_PER_CORE, C, H, W],
                                  mybir.dt.float32)
        dummy = nc.dram_tensor("t_dummy", [1, 16], mybir.dt.float32r,
                               kind="ExternalOutput")
    else:
        x_dram = nc.dram_tensor("x", [2 * C, H, W], mybir.dt.float32r,
                                kind="ExternalInput")
        out_dram = nc.dram_tensor("out", [B_PER_CORE, C, H, W],
                                  mybir.dt.float32,
                                  kind="ExternalOutput")
    x_ap = x_dram.ap()
    out_ap = out_dram.ap()

    with tile.TileContext(nc) as tc:
        with (
            tc.tile_pool(name="wpool", bufs=1) as wpool,
            tc.tile_pool(name="xs_pool", bufs=XS_BUFS) as xs_pool,
            tc.tile_pool(name="stg_pool", bufs=STG_BUFS) as stg_pool,
            tc.tile_pool(name="psum", bufs=PSUM_BUFS, space="PSUM") as psum_pool,
        ):
            wt = wpool.tile([128, 9, 128], mybir.dt.float32r)
            nc.sync.dma_start(wt[:], wt_dram.ap()[:])

            # dummy matmuls over the weight tile while the first strip
            # loads: ramps the PE clock (HAM) so the real matmuls start at
            # full speed instead of paying the cold-clock penalty
            if WARMUP_MMS:
                wt_flat = wt[:].rearrange("p t m -> p (t m)")
                psw = psum_pool.tile([128, 512], mybir.dt.float32, tag="ps")
                for i in range(WARMUP_MMS):
                    nc.tensor.matmul(psw[:], wt[:, 0, :], wt_flat[:, 0:512],
                                     start=(i == 0),
                                     stop=(i == WARMUP_MMS - 1))

            # one-time zeroing of the pad regions of each xs slot: the
            # left/right pad columns (never written by any load), and the
            # rows beyond what the smallest strip writes (guard rows for AP
            # spill + rows only larger strips own; loads rewrite the rest
            # every strip, and strips are sized so no slot ever shrinks in
            # a way that exposes stale rows within one rep).
            for _ in range(XS_BUFS):
                xi = xs_pool.tile([128, XS_ROWS, VP], mybir.dt.float32r,
                                  tag="xs")
                xif = xi[:].bitcast(mybir.dt.float32)
                nc.gpsimd.memset(xif[:, :, 0:2], 0.0)
                nc.gpsimd.memset(xif[:, :, VP - 2:VP], 0.0)
                nc.gpsimd.memset(xif[:, MIN_STRIP + 2:XS_ROWS, :], 0.0)

            for _ in range(reps):
                for s, (h0, srows, chunk_plan, out_bounds) in \
                        enumerate(STRIP_LIST):
                    xs = xs_pool.tile([128, XS_ROWS, VP], mybir.dt.float32r,
                                      tag="xs")
                    xsf = xs[:].bitcast(mybir.dt.float32)
                    # image-boundary rows the loads below leave unwritten
                    # (stale from the slot's previous strip): re-zero, the
                    # loads then overwrite whatever they do own.
                    if s == 0:
                        nc.gpsimd.memset(xsf[:, 0:2, :], 0.0)
                    if s == len(STRIP_LIST) - 1:
                        nc.gpsimd.memset(xsf[:, srows:srows + 2, :], 0.0)
                    # one load per shift group (both samples at once; the
                    # host packs both samples' group channels into one
                    # contiguous partition block); alternate the two HWDGE
                    # rings (SP / ACT) so descriptor prep runs in parallel
                    for g in range(5):
                        p0, gs2 = GROUP_P0[g], 2 * GROUP_SIZES[g]
                        dy, dx = DYS[g], DXS[g]
                        u_lo = max(h0, 1 + dy)
                        u_hi = min(h0 + srows + 1, 160 + dy)
                        eng = nc.sync if g % 2 == 0 else nc.scalar
                        eng.dma_start(
                            xs[p0:p0 + gs2,
                               u_lo - h0:u_hi - h0 + 1,
                               1 + dx:161 + dx],
                            x_ap[p0:p0 + gs2,
                                 u_lo - 1 - dy:u_hi - dy, :],
                        )
                    xs_flat = xs[:].rearrange("p r v -> p (r v)")
                    stg = stg_pool.tile([128, MAX_STRIP, W],
                                        mybir.dt.float32)
                    r0 = 0
                    for j, crows in enumerate(chunk_plan):
                        n_mm = VP * crows
                        ps = psum_pool.tile([128, 512], mybir.dt.float32,
                                            tag="ps")
                        for t, (kh, kw) in enumerate(TAPS):
                            base = (r0 + kh) * VP + kw
                            nc.tensor.matmul(
                                ps[:, 0:n_mm],
                                wt[:, t, :],
                                xs_flat[:, base:base + n_mm],
                                start=(t == 0),
                                stop=(t == len(TAPS) - 1),
                            )
                        ps_view = ps[:, 0:n_mm].rearrange(
                            "p (r v) -> p r v", v=VP)
                        ceng = getattr(
                            nc, COPY_ENGINES[j % len(COPY_ENGINES)])
                        ceng.tensor_copy(
                            stg[:, r0:r0 + crows, :],
                            ps_view[:, :, 0:W],
                        )
                        r0 += crows
                        # store each block as soon as its chunks are copied
                        # (SWDGE so stores can't head-of-line-block the
                        # HWDGE input loads)
                        if r0 in out_bounds:
                            rb = ([0] + [b for b in out_bounds if b < r0])[-1]
                            for sm in range(B_PER_CORE):
                                nc.gpsimd.dma_start(
                                    out_ap[sm, :, h0 + rb:h0 + r0, :],
                                    stg[64 * sm:64 * sm + 64, rb:r0, :],
                                )
            if timing:
                nc.sync.dma_start(dummy.ap()[:], wt[0:1, 0, 0:16])
    nc.compile()
    return nc


def _round_fp32r(a: np.ndarray) -> np.ndarray:
    b = np.ascontiguousarray(a, dtype=np.float32).view(np.uint32)
    br = (b + np.uint32(0x800)) & np.uint32(0xFFFFF000)
    return br.view(np.float32)


def _host_inputs(x: np.ndarray, weight: np.ndarray):
    """Pack channels per shift group (both samples interleaved) and build
    the sample-block-masked stationary tap matrices."""
    xv = x.reshape(N_CORES, B_PER_CORE, C, H, W)
    # x_packed[k, p] = x[2k + sample(p), channel(p)]
    x_packed = _round_fp32r(xv[:, PART_SAMPLE, PART_CHANNEL])
    # lhsT[p, t, m] = weight[co(m), channel(p), t] iff sample(p)==sample(m)
    wk = weight.transpose(1, 2, 3, 0).reshape(C, 9, C)  # [ci, tap, co]
    wt_host = np.zeros((128, 9, 128), np.float32)
    for p in range(128):
        sm = PART_SAMPLE[p]
        wt_host[p, :, 64 * sm:64 * sm + 64] = wk[PART_CHANNEL[p]]
    wt_host = _round_fp32r(wt_host)
    return x_packed, wt_host


_NC_CACHE = {}


def _get_nc(reps: int = 1):
    if reps not in _NC_CACHE:
        _NC_CACHE[reps] = build_kernel(reps)
    return _NC_CACHE[reps]


def kernel(x: np.ndarray, weight: np.ndarray) -> np.ndarray:
    x = np.asarray(x, dtype=np.float32)
    weight = np.asarray(weight, dtype=np.float32)
    x_packed, wt_host = _host_inputs(x, weight)
    nc = _get_nc(1)
    in_maps = [
        {"x": np.ascontiguousarray(x_packed[k]), "wt": wt_host}
        for k in range(N_CORES)
    ]
    res = run_bass_kernel_spmd(nc, in_maps, core_ids=list(range(N_CORES)))
    out = np.empty((B, C, H, W), np.float32)
    for k in range(N_CORES):
        out[k * B_PER_CORE:(k + 1) * B_PER_CORE] = res.results[k]["out"]
    return out


# revision 42
# speedup vs baseline: 1.0114x; 1.0114x over previous
"""Trainium2 Bass kernel for nn_DConv (shift-gather + 3x3 conv), 8 NeuronCores.

Math: the reference's per-channel torch.roll on the zero-padded image only
ever wraps in zero-pad rows/columns, so the whole op collapses to

    out[b,co,h,w] = sum_{ci,kh,kw} W[co,ci,kh,kw] * x[b,ci, h+kh-1-dy[ci], w+kw-1-dx[ci]]

with out-of-range x treated as 0 and (dy,dx) the c%5 shift table.  On device
we materialise a per-channel shifted + padded copy Xs[ci, u, v] (u,v in
[0,162)) in SBUF via DMA offsets, then run the 3x3 conv as 9 accumulating
PE matmuls over flat windows of Xs (input and output both at row pitch 162,
so each tap is a constant flat offset).

Sharding: data-parallel over batch, 2 samples per core.  SBUF partitions
hold both samples' channels grouped by shift group and interleaved
(g0s0|g0s1|g1s0|...), so each shift group loads both samples in one DMA;
each matmul uses a sample-block-masked [128,128] stationary weight so one
instruction computes the tap for both samples (K=128, M=128 -> full PE
array; out partitions 0-63 = sample 0, 64-127 = sample 1).

Dtype: float32r (fp32 with 11-bit mantissa, PE streams 1 col/cycle at
N>=256) with fp32 PSUM accumulation.  Inputs are pre-rounded to f32r on the
host so host and HW agree bit-exactly.
"""
import numpy as np

from concourse import bacc, tile, mybir
from concourse.bass_utils import run_bass_kernel_spmd

# problem shape (hardcoded per contract)
B, C, H, W = 16, 64, 160, 160
N_CORES = 8
B_PER_CORE = B // N_CORES  # 2
VP = H + 2  # padded pitch 162

# shift table: group g = ci % 5
DXS = [0, 1, 0, -1, 0]
DYS = [0, 0, 1, 0, -1]
# partition layout: for each group g (channels ci%5==g), a block of
# 2*gs partitions: sample0's gs channels then sample1's
GROUP_SIZES = [13, 13, 13, 13, 12]
GROUP_P0 = [0, 26, 52, 78, 104]   # 2 * cumulative offsets

# partition p -> (sample, channel) map, shared by host packing and weights
PART_SAMPLE = np.zeros(128, np.int64)
PART_CHANNEL = np.zeros(128, np.int64)
for _g in range(5):
    _gs = GROUP_SIZES[_g]
    for _sm in range(2):
        for _j in range(_gs):
            _p = GROUP_P0[_g] + _sm * _gs + _j
            PART_SAMPLE[_p] = _sm
            PART_CHANNEL[_p] = 5 * _j + _g

# tiling: variable strips — a small first strip shortens the initial load
# the PE has to wait for; the last strip stores in finer blocks so the
# kernel tail is short.  Each strip entry is (h0, rows, chunk_plan,
# store_bounds): chunk_plan gives output rows per PSUM chunk (each
# N = rows*162 must be in [256, 512] so f32r streams at 1 col/cycle and
# fits one PSUM bank); store_bounds are chunk-aligned row offsets where
# store DMAs fire.
_STD_PLAN = [3, 3, 3, 3, 3, 3, 2] * 2
_LAST_PLAN = [3, 3, 3, 3, 3, 3, 2] + [2, 2, 3, 3] + [3, 3, 2, 2]
STRIP_LIST = [
    (0, 40, _STD_PLAN, (20, 40)),
    (40, 40, _STD_PLAN, (20, 40)),
    (80, 40, _STD_PLAN, (20, 40)),
    (120, 40, _LAST_PLAN, (20, 30, 36, 40)),
]
MAX_STRIP = max(r for _, r, _, _ in STRIP_LIST)     # 48
MIN_STRIP = min(r for _, r, _, _ in STRIP_LIST)     # 24
XS_ROWS = MAX_STRIP + 3    # strip buffer rows (+1 halo each side +1 guard)

XS_BUFS = 3
STG_BUFS = 2
PSUM_BUFS = 4
WARMUP_MMS = 16            # dummy matmuls to ramp the PE during the first load
COPY_ENGINES = ("vector",)  # round-robin engines for PSUM->SBUF copies

TAPS = [(kh, kw) for kh in range(3) for kw in range(3)]


def build_kernel(reps: int = 1, timing: bool = False):
    nc = bacc.Bacc("TRN2", target_bir_lowering=False, debug=False,
                   num_devices=N_CORES)
    wt_dram = nc.dram_tensor("wt", [128, 9, 128], mybir.dt.float32r,
                             kind="ExternalInput")
    if timing:
        # timing-only variant: big tensors stay in device DRAM (uninitialised
        # garbage is fine for timing) so per-call host<->device transfer is
        # tiny and wall-clock noise is dominated by the fixed RTT only.
        x_dram = nc.dram_tensor("x", [2 * C, H, W], mybir.dt.float32r)
        out_dram = nc.dram_tensor("out", [B_PER_CORE, C, H, W],
                                  mybir.dt.float32)
        dummy = nc.dram_tensor("t_dummy", [1, 16], mybir.dt.float32r,
                               kind="ExternalOutput")
    else:
        x_dram = nc.dram_tensor("x", [2 * C, H, W], mybir.dt.float32r,
                                kind="ExternalInput")
        out_dram = nc.dram_tensor("out", [B_PER_CORE, C, H, W],
                                  mybir.dt.float32,
                                  kind="ExternalOutput")
    x_ap = x_dram.ap()
    out_flat = out_dram.ap().rearrange("b c h w -> (b c) h w")

    with tile.TileContext(nc) as tc:
        with (
            tc.tile_pool(name="wpool", bufs=1) as wpool,
            tc.tile_pool(name="xs_pool", bufs=XS_BUFS) as xs_pool,
            tc.tile_pool(name="stg_pool", bufs=STG_BUFS) as stg_pool,
            tc.tile_pool(name="psum", bufs=PSUM_BUFS, space="PSUM") as psum_pool,
        ):
            wt = wpool.tile([128, 9, 128], mybir.dt.float32r)
            nc.sync.dma_start(wt[:], wt_dram.ap()[:])

            # dummy matmuls over the weight tile while the first strip
            # loads: ramps the PE clock (HAM) so the real matmuls start at
            # full speed instead of paying the cold-clock penalty (they are
            # sized to end just after the first strip's loads land, keeping
            # the PE continuously busy into the real matmul stream)
            if WARMUP_MMS:
                wt_flat = wt[:].rearrange("p t m -> p (t m)")
                psw = psum_pool.tile([128, 512], mybir.dt.float32, tag="ps")
                for i in range(WARMUP_MMS):
                    nc.tensor.matmul(psw[:], wt[:, 0, :], wt_flat[:, 0:512],
                                     start=(i == 0),
                                     stop=(i == WARMUP_MMS - 1))

            # one-time zeroing of the pad regions of each xs slot: the
            # left/right pad columns (never written by any load), and the
            # rows beyond what the smallest strip writes (guard rows for AP
            # spill + rows only larger strips own; loads rewrite the rest
            # every strip, and strips are sized so no slot ever shrinks in
            # a way that exposes stale rows within one rep).
            for _ in range(XS_BUFS):
                xi = xs_pool.tile([128, XS_ROWS, VP], mybir.dt.float32r,
                                  tag="xs")
                xif = xi[:].bitcast(mybir.dt.float32)
                nc.gpsimd.memset(xif[:, :, 0:2], 0.0)
                nc.gpsimd.memset(xif[:, :, VP - 2:VP], 0.0)
                nc.gpsimd.memset(xif[:, MIN_STRIP + 2:XS_ROWS, :], 0.0)

            for _ in range(reps):
                for s, (h0, srows, chunk_plan, out_bounds) in \
                        enumerate(STRIP_LIST):
                    xs = xs_pool.tile([128, XS_ROWS, VP], mybir.dt.float32r,
                                      tag="xs")
                    xsf = xs[:].bitcast(mybir.dt.float32)
                    # image-boundary rows the loads below leave unwritten
                    # (stale from the slot's previous strip): re-zero, the
                    # loads then overwrite whatever they do own.
                    if s == 0:
                        nc.gpsimd.memset(xsf[:, 0:2, :], 0.0)
                    if s == len(STRIP_LIST) - 1:
                        nc.gpsimd.memset(xsf[:, srows:srows + 2, :], 0.0)
                    # one load per shift group (both samples at once; the
                    # host packs both samples' group channels into one
                    # contiguous partition block); alternate the two HWDGE
                    # rings (SP / ACT) so descriptor prep runs in parallel
                    for g in range(5):
                        p0, gs2 = GROUP_P0[g], 2 * GROUP_SIZES[g]
                        dy, dx = DYS[g], DXS[g]
                        u_lo = max(h0, 1 + dy)
                        u_hi = min(h0 + srows + 1, 160 + dy)
                        eng = nc.sync if g % 2 == 0 else nc.scalar
                        eng.dma_start(
                            xs[p0:p0 + gs2,
                               u_lo - h0:u_hi - h0 + 1,
                               1 + dx:161 + dx],
                            x_ap[p0:p0 + gs2,
                                 u_lo - 1 - dy:u_hi - dy, :],
                        )
                    xs_flat = xs[:].rearrange("p r v -> p (r v)")
                    stg = stg_pool.tile([128, MAX_STRIP, W],
                                        mybir.dt.float32)
                    r0 = 0
                    for j, crows in enumerate(chunk_plan):
                        n_mm = VP * crows
                        ps = psum_pool.tile([128, 512], mybir.dt.float32,
                                            tag="ps")
                        for t, (kh, kw) in enumerate(TAPS):
                            base = (r0 + kh) * VP + kw
                            nc.tensor.matmul(
                                ps[:, 0:n_mm],
                                wt[:, t, :],
                                xs_flat[:, base:base + n_mm],
                                start=(t == 0),
                                stop=(t == len(TAPS) - 1),
                            )
                        ps_view = ps[:, 0:n_mm].rearrange(
                            "p (r v) -> p r v", v=VP)
                        ceng = getattr(
                            nc, COPY_ENGINES[j % len(COPY_ENGINES)])
                        ceng.tensor_copy(
                            stg[:, r0:r0 + crows, :],
                            ps_view[:, :, 0:W],
                        )
                        r0 += crows
                        # store each block as soon as its chunks are copied
                        # (SWDGE so stores can't head-of-line-block the
                        # HWDGE input loads).  One DMA covers both samples:
                        # the HBM (b c) dims are contiguous, matching the
                        # 128-partition staging layout.
                        if r0 in out_bounds:
                            rb = ([0] + [b for b in out_bounds if b < r0])[-1]
                            nc.gpsimd.dma_start(
                                out_flat[:, h0 + rb:h0 + r0, :],
                                stg[:, rb:r0, :],
                            )
            if timing:
                nc.sync.dma_start(dummy.ap()[:], wt[0:1, 0, 0:16])
    nc.compile()
    return nc


def _round_fp32r(a: np.ndarray) -> np.ndarray:
    b = np.ascontiguousarray(a, dtype=np.float32).view(np.uint32)
    br = (b + np.uint32(0x800)) & np.uint32(0xFFFFF000)
    return br.view(np.float32)


def _host_inputs(x: np.ndarray, weight: np.ndarray):
    """Pack channels per shift group (both samples interleaved) and build
    the sample-block-masked stationary tap matrices."""
    xv = x.reshape(N_CORES, B_PER_CORE, C, H, W)
    # x_packed[k, p] = x[2k + sample(p), channel(p)]
    x_packed = _round_fp32r(xv[:, PART_SAMPLE, PART_CHANNEL])
    # lhsT[p, t, m] = weight[co(m), channel(p), t] iff sample(p)==sample(m)
    wk = weight.transpose(1, 2, 3, 0).reshape(C, 9, C)  # [ci, tap, co]
    wt_host = np.zeros((128, 9, 128), np.float32)
    for p in range(128):
        sm = PART_SAMPLE[p]
        wt_host[p, :, 64 * sm:64 * sm + 64] = wk[PART_CHANNEL[p]]
    wt_host = _round_fp32r(wt_host)
    return x_packed, wt_host


_NC_CACHE = {}


def _get_nc(reps: int = 1):
    if reps not in _NC_CACHE:
        _NC_CACHE[reps] = build_kernel(reps)
    return _NC_CACHE[reps]


def kernel(x: np.ndarray, weight: np.ndarray) -> np.ndarray:
    x = np.asarray(x, dtype=np.float32)
    weight = np.asarray(weight, dtype=np.float32)
    x_packed, wt_host = _host_inputs(x, weight)
    nc = _get_nc(1)
    in_maps = [
        {"x": np.ascontiguousarray(x_packed[k]), "wt": wt_host}
        for k in range(N_CORES)
    ]
    res = run_bass_kernel_spmd(nc, in_maps, core_ids=list(range(N_CORES)))
    out = np.empty((B, C, H, W), np.float32)
    for k in range(N_CORES):
        out[k * B_PER_CORE:(k + 1) * B_PER_CORE] = res.results[k]["out"]
    return out


# revision 43
# speedup vs baseline: 1.0149x; 1.0034x over previous
"""Trainium2 Bass kernel for nn_DConv (shift-gather + 3x3 conv), 8 NeuronCores.

Math: the reference's per-channel torch.roll on the zero-padded image only
ever wraps in zero-pad rows/columns, so the whole op collapses to

    out[b,co,h,w] = sum_{ci,kh,kw} W[co,ci,kh,kw] * x[b,ci, h+kh-1-dy[ci], w+kw-1-dx[ci]]

with out-of-range x treated as 0 and (dy,dx) the c%5 shift table.  On device
we materialise a per-channel shifted + padded copy Xs[ci, u, v] (u,v in
[0,162)) in SBUF via DMA offsets, then run the 3x3 conv as 9 accumulating
PE matmuls over flat windows of Xs (input and output both at row pitch 162,
so each tap is a constant flat offset).

Sharding: data-parallel over batch, 2 samples per core.  SBUF partitions
hold both samples' channels grouped by shift group and interleaved
(g0s0|g0s1|g1s0|...), so each shift group loads both samples in one DMA;
each matmul uses a sample-block-masked [128,128] stationary weight so one
instruction computes the tap for both samples (K=128, M=128 -> full PE
array; out partitions 0-63 = sample 0, 64-127 = sample 1).

Dtype: float32r (fp32 with 11-bit mantissa, PE streams 1 col/cycle at
N>=256) with fp32 PSUM accumulation.  Inputs are pre-rounded to f32r on the
host so host and HW agree bit-exactly.
"""
import numpy as np

from concourse import bacc, tile, mybir
from concourse.bass_utils import run_bass_kernel_spmd

# problem shape (hardcoded per contract)
B, C, H, W = 16, 64, 160, 160
N_CORES = 8
B_PER_CORE = B // N_CORES  # 2
VP = H + 2  # padded pitch 162

# shift table: group g = ci % 5
DXS = [0, 1, 0, -1, 0]
DYS = [0, 0, 1, 0, -1]
# partition layout: for each group g (channels ci%5==g), a block of
# 2*gs partitions: sample0's gs channels then sample1's
GROUP_SIZES = [13, 13, 13, 13, 12]
GROUP_P0 = [0, 26, 52, 78, 104]   # 2 * cumulative offsets

# partition p -> (sample, channel) map, shared by host packing and weights
PART_SAMPLE = np.zeros(128, np.int64)
PART_CHANNEL = np.zeros(128, np.int64)
for _g in range(5):
    _gs = GROUP_SIZES[_g]
    for _sm in range(2):
        for _j in range(_gs):
            _p = GROUP_P0[_g] + _sm * _gs + _j
            PART_SAMPLE[_p] = _sm
            PART_CHANNEL[_p] = 5 * _j + _g

# tiling: variable strips — a small first strip shortens the initial load
# the PE has to wait for; the last strip stores in finer blocks so the
# kernel tail is short.  Each strip entry is (h0, rows, chunk_plan,
# store_bounds): chunk_plan gives output rows per PSUM chunk (each
# N = rows*162 must be in [256, 512] so f32r streams at 1 col/cycle and
# fits one PSUM bank); store_bounds are chunk-aligned row offsets where
# store DMAs fire.
_STD_PLAN = [3, 3, 3, 3, 3, 3, 2] * 2
_LAST_PLAN = [3, 3, 3, 3, 3, 3, 2] + [2, 2, 3, 3] + [3, 3, 2, 2]
STRIP_LIST = [
    (0, 40, _STD_PLAN, (20, 40)),
    (40, 40, _STD_PLAN, (20, 40)),
    (80, 40, _STD_PLAN, (20, 40)),
    (120, 40, _LAST_PLAN, (20, 30, 36, 38, 40)),
]
MAX_STRIP = max(r for _, r, _, _ in STRIP_LIST)     # 48
MIN_STRIP = min(r for _, r, _, _ in STRIP_LIST)     # 24
XS_ROWS = MAX_STRIP + 3    # strip buffer rows (+1 halo each side +1 guard)

XS_BUFS = 3
STG_BUFS = 2
PSUM_BUFS = 4
WARMUP_MMS = 16            # dummy matmuls to ramp the PE during the first load
COPY_ENGINES = ("vector",)  # round-robin engines for PSUM->SBUF copies

TAPS = [(kh, kw) for kh in range(3) for kw in range(3)]


def build_kernel(reps: int = 1, timing: bool = False):
    nc = bacc.Bacc("TRN2", target_bir_lowering=False, debug=False,
                   num_devices=N_CORES)
    wt_dram = nc.dram_tensor("wt", [128, 9, 128], mybir.dt.float32r,
                             kind="ExternalInput")
    if timing:
        # timing-only variant: big tensors stay in device DRAM (uninitialised
        # garbage is fine for timing) so per-call host<->device transfer is
        # tiny and wall-clock noise is dominated by the fixed RTT only.
        x_dram = nc.dram_tensor("x", [2 * C, H, W], mybir.dt.float32r)
        out_dram = nc.dram_tensor("out", [B_PER_CORE, C, H, W],
                                  mybir.dt.float32)
        dummy = nc.dram_tensor("t_dummy", [1, 16], mybir.dt.float32r,
                               kind="ExternalOutput")
    else:
        x_dram = nc.dram_tensor("x", [2 * C, H, W], mybir.dt.float32r,
                                kind="ExternalInput")
        out_dram = nc.dram_tensor("out", [B_PER_CORE, C, H, W],
                                  mybir.dt.float32,
                                  kind="ExternalOutput")
    x_ap = x_dram.ap()
    out_flat = out_dram.ap().rearrange("b c h w -> (b c) h w")

    with tile.TileContext(nc) as tc:
        with (
            tc.tile_pool(name="wpool", bufs=1) as wpool,
            tc.tile_pool(name="xs_pool", bufs=XS_BUFS) as xs_pool,
            tc.tile_pool(name="stg_pool", bufs=STG_BUFS) as stg_pool,
            tc.tile_pool(name="psum", bufs=PSUM_BUFS, space="PSUM") as psum_pool,
        ):
            wt = wpool.tile([128, 9, 128], mybir.dt.float32r)
            nc.sync.dma_start(wt[:], wt_dram.ap()[:])

            # dummy matmuls over the weight tile while the first strip
            # loads: ramps the PE clock (HAM) so the real matmuls start at
            # full speed instead of paying the cold-clock penalty (they are
            # sized to end just after the first strip's loads land, keeping
            # the PE continuously busy into the real matmul stream)
            if WARMUP_MMS:
                wt_flat = wt[:].rearrange("p t m -> p (t m)")
                psw = psum_pool.tile([128, 512], mybir.dt.float32, tag="ps")
                for i in range(WARMUP_MMS):
                    nc.tensor.matmul(psw[:], wt[:, 0, :], wt_flat[:, 0:512],
                                     start=(i == 0),
                                     stop=(i == WARMUP_MMS - 1))

            # one-time zeroing of the pad regions of each xs slot: the
            # left/right pad columns (never written by any load), and the
            # rows beyond what the smallest strip writes (guard rows for AP
            # spill + rows only larger strips own; loads rewrite the rest
            # every strip, and strips are sized so no slot ever shrinks in
            # a way that exposes stale rows within one rep).
            for _ in range(XS_BUFS):
                xi = xs_pool.tile([128, XS_ROWS, VP], mybir.dt.float32r,
                                  tag="xs")
                xif = xi[:].bitcast(mybir.dt.float32)
                nc.gpsimd.memset(xif[:, :, 0:2], 0.0)
                nc.gpsimd.memset(xif[:, :, VP - 2:VP], 0.0)
                nc.gpsimd.memset(xif[:, MIN_STRIP + 2:XS_ROWS, :], 0.0)

            for _ in range(reps):
                for s, (h0, srows, chunk_plan, out_bounds) in \
                        enumerate(STRIP_LIST):
                    xs = xs_pool.tile([128, XS_ROWS, VP], mybir.dt.float32r,
                                      tag="xs")
                    xsf = xs[:].bitcast(mybir.dt.float32)
                    # image-boundary rows the loads below leave unwritten
                    # (stale from the slot's previous strip): re-zero, the
                    # loads then overwrite whatever they do own.
                    if s == 0:
                        nc.gpsimd.memset(xsf[:, 0:2, :], 0.0)
                    if s == len(STRIP_LIST) - 1:
                        nc.gpsimd.memset(xsf[:, srows:srows + 2, :], 0.0)
                    # one load per shift group (both samples at once; the
                    # host packs both samples' group channels into one
                    # contiguous partition block); alternate the two HWDGE
                    # rings (SP / ACT) so descriptor prep runs in parallel
                    for g in range(5):
                        p0, gs2 = GROUP_P0[g], 2 * GROUP_SIZES[g]
                        dy, dx = DYS[g], DXS[g]
                        u_lo = max(h0, 1 + dy)
                        u_hi = min(h0 + srows + 1, 160 + dy)
                        eng = nc.sync if g % 2 == 0 else nc.scalar
                        eng.dma_start(
                            xs[p0:p0 + gs2,
                               u_lo - h0:u_hi - h0 + 1,
                               1 + dx:161 + dx],
                            x_ap[p0:p0 + gs2,
                                 u_lo - 1 - dy:u_hi - dy, :],
                        )
                    xs_flat = xs[:].rearrange("p r v -> p (r v)")
                    stg = stg_pool.tile([128, MAX_STRIP, W],
                                        mybir.dt.float32)
                    r0 = 0
                    for j, crows in enumerate(chunk_plan):
                        n_mm = VP * crows
                        ps = psum_pool.tile([128, 512], mybir.dt.float32,
                                            tag="ps")
                        for t, (kh, kw) in enumerate(TAPS):
                            base = (r0 + kh) * VP + kw
                            nc.tensor.matmul(
                                ps[:, 0:n_mm],
                                wt[:, t, :],
                                xs_flat[:, base:base + n_mm],
                                start=(t == 0),
                                stop=(t == len(TAPS) - 1),
                            )
                        ps_view = ps[:, 0:n_mm].rearrange(
                            "p (r v) -> p r v", v=VP)
                        ceng = getattr(
                            nc, COPY_ENGINES[j % len(COPY_ENGINES)])
                        ceng.tensor_copy(
                            stg[:, r0:r0 + crows, :],
                            ps_view[:, :, 0:W],
                        )
                        r0 += crows
                        # store each block as soon as its chunks are copied
                        # (SWDGE so stores can't head-of-line-block the
                        # HWDGE input loads).  One DMA covers both samples:
                        # the HBM (b c) dims are contiguous, matching the
                        # 128-partition staging layout.
                        if r0 in out_bounds:
                            rb = ([0] + [b for b in out_bounds if b < r0])[-1]
                            nc.gpsimd.dma_start(
                                out_flat[:, h0 + rb:h0 + r0, :],
                                stg[:, rb:r0, :],
                            )
            if timing:
                nc.sync.dma_start(dummy.ap()[:], wt[0:1, 0, 0:16])
    nc.compile()
    return nc


def _round_fp32r(a: np.ndarray) -> np.ndarray:
    b = np.ascontiguousarray(a, dtype=np.float32).view(np.uint32)
    br = (b + np.uint32(0x800)) & np.uint32(0xFFFFF000)
    return br.view(np.float32)


def _host_inputs(x: np.ndarray, weight: np.ndarray):
    """Pack channels per shift group (both samples interleaved) and build
    the sample-block-masked stationary tap matrices."""
    xv = x.reshape(N_CORES, B_PER_CORE, C, H, W)
    # x_packed[k, p] = x[2k + sample(p), channel(p)]
    x_packed = _round_fp32r(xv[:, PART_SAMPLE, PART_CHANNEL])
    # lhsT[p, t, m] = weight[co(m), channel(p), t] iff sample(p)==sample(m)
    wk = weight.transpose(1, 2, 3, 0).reshape(C, 9, C)  # [ci, tap, co]
    wt_host = np.zeros((128, 9, 128), np.float32)
    for p in range(128):
        sm = PART_SAMPLE[p]
        wt_host[p, :, 64 * sm:64 * sm + 64] = wk[PART_CHANNEL[p]]
    wt_host = _round_fp32r(wt_host)
    return x_packed, wt_host


_NC_CACHE = {}


def _get_nc(reps: int = 1):
    if reps not in _NC_CACHE:
        _NC_CACHE[reps] = build_kernel(reps)
    return _NC_CACHE[reps]


def kernel(x: np.ndarray, weight: np.ndarray) -> np.ndarray:
    x = np.asarray(x, dtype=np.float32)
    weight = np.asarray(weight, dtype=np.float32)
    x_packed, wt_host = _host_inputs(x, weight)
    nc = _get_nc(1)
    in_maps = [
        {"x": np.ascontiguousarray(x_packed[k]), "wt": wt_host}
        for k in range(N_CORES)
    ]
    res = run_bass_kernel_spmd(nc, in_maps, core_ids=list(range(N_CORES)))
    out = np.empty((B, C, H, W), np.float32)
    for k in range(N_CORES):
        out[k * B_PER_CORE:(k + 1) * B_PER_CORE] = res.results[k]["out"]
    return out


# revision 44
# speedup vs baseline: 1.0166x; 1.0017x over previous
"""Trainium2 Bass kernel for nn_DConv (shift-gather + 3x3 conv), 8 NeuronCores.

Math: the reference's per-channel torch.roll on the zero-padded image only
ever wraps in zero-pad rows/columns, so the whole op collapses to

    out[b,co,h,w] = sum_{ci,kh,kw} W[co,ci,kh,kw] * x[b,ci, h+kh-1-dy[ci], w+kw-1-dx[ci]]

with out-of-range x treated as 0 and (dy,dx) the c%5 shift table.  On device
we materialise a per-channel shifted + padded copy Xs[ci, u, v] (u,v in
[0,162)) in SBUF via DMA offsets, then run the 3x3 conv as 9 accumulating
PE matmuls over flat windows of Xs (input and output both at row pitch 162,
so each tap is a constant flat offset).

Sharding: data-parallel over batch, 2 samples per core.  SBUF partitions
hold both samples' channels grouped by shift group and interleaved
(g0s0|g0s1|g1s0|...), so each shift group loads both samples in one DMA;
each matmul uses a sample-block-masked [128,128] stationary weight so one
instruction computes the tap for both samples (K=128, M=128 -> full PE
array; out partitions 0-63 = sample 0, 64-127 = sample 1).

Dtype: float32r (fp32 with 11-bit mantissa, PE streams 1 col/cycle at
N>=256) with fp32 PSUM accumulation.  Inputs are pre-rounded to f32r on the
host so host and HW agree bit-exactly.
"""
import numpy as np

from concourse import bacc, tile, mybir
from concourse.bass_utils import run_bass_kernel_spmd

# problem shape (hardcoded per contract)
B, C, H, W = 16, 64, 160, 160
N_CORES = 8
B_PER_CORE = B // N_CORES  # 2
VP = H + 2  # padded pitch 162

# shift table: group g = ci % 5
DXS = [0, 1, 0, -1, 0]
DYS = [0, 0, 1, 0, -1]
# partition layout: for each group g (channels ci%5==g), a block of
# 2*gs partitions: sample0's gs channels then sample1's
GROUP_SIZES = [13, 13, 13, 13, 12]
GROUP_P0 = [0, 26, 52, 78, 104]   # 2 * cumulative offsets

# partition p -> (sample, channel) map, shared by host packing and weights
PART_SAMPLE = np.zeros(128, np.int64)
PART_CHANNEL = np.zeros(128, np.int64)
for _g in range(5):
    _gs = GROUP_SIZES[_g]
    for _sm in range(2):
        for _j in range(_gs):
            _p = GROUP_P0[_g] + _sm * _gs + _j
            PART_SAMPLE[_p] = _sm
            PART_CHANNEL[_p] = 5 * _j + _g

# tiling: variable strips — a small first strip shortens the initial load
# the PE has to wait for; the last strip stores in finer blocks so the
# kernel tail is short.  Each strip entry is (h0, rows, chunk_plan,
# store_bounds): chunk_plan gives output rows per PSUM chunk (each
# N = rows*162 must be in [256, 512] so f32r streams at 1 col/cycle and
# fits one PSUM bank); store_bounds are chunk-aligned row offsets where
# store DMAs fire.
_STD_PLAN = [3, 3, 3, 3, 3, 3, 2] * 2
_LAST_PLAN = [3, 3, 3, 3, 3, 3, 2] + [2, 2, 3, 3] + [3, 3, 2, 2]
STRIP_LIST = [
    (0, 40, _STD_PLAN, (20, 40)),
    (40, 40, _STD_PLAN, (20, 40)),
    (80, 40, _STD_PLAN, (20, 40)),
    (120, 40, _LAST_PLAN, (20, 30, 36, 38, 40)),
]
MAX_STRIP = max(r for _, r, _, _ in STRIP_LIST)     # 48
MIN_STRIP = min(r for _, r, _, _ in STRIP_LIST)     # 24
XS_ROWS = MAX_STRIP + 3    # strip buffer rows (+1 halo each side +1 guard)

XS_BUFS = 3
STG_BUFS = 2
PSUM_BUFS = 4
WARMUP_MMS = 16            # dummy matmuls to ramp the PE during the first load
COPY_ENGINES = ("vector",)  # round-robin engines for PSUM->SBUF copies

TAPS = [(kh, kw) for kh in range(3) for kw in range(3)]


def build_kernel(reps: int = 1, timing: bool = False):
    nc = bacc.Bacc("TRN2", target_bir_lowering=False, debug=False,
                   num_devices=N_CORES)
    wt_dram = nc.dram_tensor("wt", [128, 9, 128], mybir.dt.float32r,
                             kind="ExternalInput")
    if timing:
        # timing-only variant: big tensors stay in device DRAM (uninitialised
        # garbage is fine for timing) so per-call host<->device transfer is
        # tiny and wall-clock noise is dominated by the fixed RTT only.
        x_dram = nc.dram_tensor("x", [2 * C, H, W], mybir.dt.float32r)
        out_dram = nc.dram_tensor("out", [B_PER_CORE, C, H, W],
                                  mybir.dt.float32)
        dummy = nc.dram_tensor("t_dummy", [1, 16], mybir.dt.float32r,
                               kind="ExternalOutput")
    else:
        x_dram = nc.dram_tensor("x", [2 * C, H, W], mybir.dt.float32r,
                                kind="ExternalInput")
        out_dram = nc.dram_tensor("out", [B_PER_CORE, C, H, W],
                                  mybir.dt.float32,
                                  kind="ExternalOutput")
    x_ap = x_dram.ap()
    out_flat = out_dram.ap().rearrange("b c h w -> (b c) h w")

    with tile.TileContext(nc) as tc:
        with (
            tc.tile_pool(name="wpool", bufs=1) as wpool,
            tc.tile_pool(name="xs_pool", bufs=XS_BUFS) as xs_pool,
            tc.tile_pool(name="stg_pool", bufs=STG_BUFS) as stg_pool,
            tc.tile_pool(name="psum", bufs=PSUM_BUFS, space="PSUM") as psum_pool,
        ):
            wt = wpool.tile([128, 9, 128], mybir.dt.float32r)
            nc.sync.dma_start(wt[:], wt_dram.ap()[:])

            # dummy matmuls over the weight tile while the first strip
            # loads: ramps the PE clock (HAM) so the real matmuls start at
            # full speed instead of paying the cold-clock penalty (they are
            # sized to end just after the first strip's loads land, keeping
            # the PE continuously busy into the real matmul stream)
            if WARMUP_MMS:
                wt_flat = wt[:].rearrange("p t m -> p (t m)")
                psw = psum_pool.tile([128, 512], mybir.dt.float32, tag="ps")
                for i in range(WARMUP_MMS):
                    nc.tensor.matmul(psw[:], wt[:, 0, :], wt_flat[:, 0:512],
                                     start=(i == 0),
                                     stop=(i == WARMUP_MMS - 1))

            # one-time zeroing of the pad regions of each xs slot: the
            # left/right pad columns (never written by any load), and the
            # rows beyond what the smallest strip writes (guard rows for AP
            # spill + rows only larger strips own; loads rewrite the rest
            # every strip, and strips are sized so no slot ever shrinks in
            # a way that exposes stale rows within one rep).
            for _ in range(XS_BUFS):
                xi = xs_pool.tile([128, XS_ROWS, VP], mybir.dt.float32r,
                                  tag="xs")
                xif = xi[:].bitcast(mybir.dt.float32)
                nc.gpsimd.memset(xif[:, :, 0:2], 0.0)
                nc.gpsimd.memset(xif[:, :, VP - 2:VP], 0.0)
                nc.gpsimd.memset(xif[:, MIN_STRIP + 2:XS_ROWS, :], 0.0)

            for _ in range(reps):
                for s, (h0, srows, chunk_plan, out_bounds) in \
                        enumerate(STRIP_LIST):
                    xs = xs_pool.tile([128, XS_ROWS, VP], mybir.dt.float32r,
                                      tag="xs")
                    xsf = xs[:].bitcast(mybir.dt.float32)
                    # image-boundary rows the loads below leave unwritten
                    # (stale from the slot's previous strip): re-zero, the
                    # loads then overwrite whatever they do own.
                    if s == 0:
                        nc.gpsimd.memset(xsf[:, 0:2, :], 0.0)
                    if s == len(STRIP_LIST) - 1:
                        nc.gpsimd.memset(xsf[:, srows:srows + 2, :], 0.0)
                    # one load per shift group (both samples at once; the
                    # host packs both samples' group channels into one
                    # contiguous partition block); alternate the two HWDGE
                    # rings (SP / ACT) so descriptor prep runs in parallel
                    for g in range(5):
                        p0, gs2 = GROUP_P0[g], 2 * GROUP_SIZES[g]
                        dy, dx = DYS[g], DXS[g]
                        u_lo = max(h0, 1 + dy)
                        u_hi = min(h0 + srows + 1, 160 + dy)
                        eng = nc.sync if g % 2 == 0 else nc.scalar
                        eng.dma_start(
                            xs[p0:p0 + gs2,
                               u_lo - h0:u_hi - h0 + 1,
                               1 + dx:161 + dx],
                            x_ap[p0:p0 + gs2,
                                 u_lo - 1 - dy:u_hi - dy, :],
                        )
                    xs_flat = xs[:].rearrange("p r v -> p (r v)")
                    stg = stg_pool.tile([128, MAX_STRIP, W],
                                        mybir.dt.float32)
                    r0 = 0
                    for j, crows in enumerate(chunk_plan):
                        n_mm = VP * crows
                        ps = psum_pool.tile([128, 512], mybir.dt.float32,
                                            tag="ps")
                        for t, (kh, kw) in enumerate(TAPS):
                            base = (r0 + kh) * VP + kw
                            nc.tensor.matmul(
                                ps[:, 0:n_mm],
                                wt[:, t, :],
                                xs_flat[:, base:base + n_mm],
                                start=(t == 0),
                                stop=(t == len(TAPS) - 1),
                            )
                        ps_view = ps[:, 0:n_mm].rearrange(
                            "p (r v) -> p r v", v=VP)
                        ceng = getattr(
                            nc, COPY_ENGINES[j % len(COPY_ENGINES)])
                        ceng.tensor_copy(
                            stg[:, r0:r0 + crows, :],
                            ps_view[:, :, 0:W],
                        )
                        r0 += crows
                        # store each block as soon as its chunks are copied
                        # (SWDGE so stores can't head-of-line-block the
                        # HWDGE input loads).  One DMA covers both samples:
                        # the HBM (b c) dims are contiguous, matching the
                        # 128-partition staging layout.
                        if r0 in out_bounds:
                            rb = ([0] + [b for b in out_bounds if b < r0])[-1]
                            # last strip's stores go HWDGE (ACT): cheaper
                            # descriptor prep, and with no loads left there
                            # is nothing for them to head-of-line-block
                            seng = (nc.scalar if s == len(STRIP_LIST) - 1
                                    else nc.gpsimd)
                            seng.dma_start(
                                out_flat[:, h0 + rb:h0 + r0, :],
                                stg[:, rb:r0, :],
                            )
            if timing:
                nc.sync.dma_start(dummy.ap()[:], wt[0:1, 0, 0:16])
    nc.compile()
    return nc


def _round_fp32r(a: np.ndarray) -> np.ndarray:
    b = np.ascontiguousarray(a, dtype=np.float32).view(np.uint32)
    br = (b + np.uint32(0x800)) & np.uint32(0xFFFFF000)
    return br.view(np.float32)


def _host_inputs(x: np.ndarray, weight: np.ndarray):
    """Pack channels per shift group (both samples interleaved) and build
    the sample-block-masked stationary tap matrices."""
    xv = x.reshape(N_CORES, B_PER_CORE, C, H, W)
    # x_packed[k, p] = x[2k + sample(p), channel(p)]
    x_packed = _round_fp32r(xv[:, PART_SAMPLE, PART_CHANNEL])
    # lhsT[p, t, m] = weight[co(m), channel(p), t] iff sample(p)==sample(m)
    wk = weight.transpose(1, 2, 3, 0).reshape(C, 9, C)  # [ci, tap, co]
    wt_host = np.zeros((128, 9, 128), np.float32)
    for p in range(128):
        sm = PART_SAMPLE[p]
        wt_host[p, :, 64 * sm:64 * sm + 64] = wk[PART_CHANNEL[p]]
    wt_host = _round_fp32r(wt_host)
    return x_packed, wt_host


_NC_CACHE = {}


def _get_nc(reps: int = 1):
    if reps not in _NC_CACHE:
        _NC_CACHE[reps] = build_kernel(reps)
    return _NC_CACHE[reps]


def kernel(x: np.ndarray, weight: np.ndarray) -> np.ndarray:
    x = np.asarray(x, dtype=np.float32)
    weight = np.asarray(weight, dtype=np.float32)
    x_packed, wt_host = _host_inputs(x, weight)
    nc = _get_nc(1)
    in_maps = [
        {"x": np.ascontiguousarray(x_packed[k]), "wt": wt_host}
        for k in range(N_CORES)
    ]
    res = run_bass_kernel_spmd(nc, in_maps, core_ids=list(range(N_CORES)))
    out = np.empty((B, C, H, W), np.float32)
    for k in range(N_CORES):
        out[k * B_PER_CORE:(k + 1) * B_PER_CORE] = res.results[k]["out"]
    return out


# revision 47
# speedup vs baseline: 1.0598x; 1.0425x over previous
"""Trainium2 Bass kernel for nn_DConv (shift-gather + 3x3 conv), 8 NeuronCores.

Math: the reference's per-channel torch.roll on the zero-padded image only
ever wraps in zero-pad rows/columns, so the whole op collapses to

    out[b,co,h,w] = sum_{ci,kh,kw} W[co,ci,kh,kw] * x[b,ci, h+kh-1-dy[ci], w+kw-1-dx[ci]]

with out-of-range x treated as 0 and (dy,dx) the c%5 shift table.  On device
we materialise a per-channel shifted + padded copy Xs[ci, u, v] (u,v in
[0,162)) in SBUF via DMA offsets, then run the 3x3 conv as 9 accumulating
PE matmuls over flat windows of Xs (input and output both at row pitch 162,
so each tap is a constant flat offset).

Sharding: data-parallel over batch, 2 samples per core.  SBUF partitions
hold both samples' channels grouped by shift group and interleaved
(g0s0|g0s1|g1s0|...), so each shift group loads both samples in one DMA;
each matmul uses a sample-block-masked [128,128] stationary weight so one
instruction computes the tap for both samples (K=128, M=128 -> full PE
array; out partitions 0-63 = sample 0, 64-127 = sample 1).

Dtype: float32r (fp32 with 11-bit mantissa, PE streams 1 col/cycle at
N>=256) with fp32 PSUM accumulation.  Inputs are pre-rounded to f32r on the
host so host and HW agree bit-exactly.
"""
import numpy as np

from concourse import bacc, tile, mybir
from concourse.bass_utils import run_bass_kernel_spmd

# problem shape (hardcoded per contract)
B, C, H, W = 16, 64, 160, 160
N_CORES = 8
B_PER_CORE = B // N_CORES  # 2
VP = H + 2  # padded pitch 162

# shift table: group g = ci % 5
DXS = [0, 1, 0, -1, 0]
DYS = [0, 0, 1, 0, -1]
# partition layout: for each group g (channels ci%5==g), a block of
# 2*gs partitions: sample0's gs channels then sample1's
GROUP_SIZES = [13, 13, 13, 13, 12]
GROUP_P0 = [0, 26, 52, 78, 104]   # 2 * cumulative offsets

# partition p -> (sample, channel) map, shared by host packing and weights
PART_SAMPLE = np.zeros(128, np.int64)
PART_CHANNEL = np.zeros(128, np.int64)
for _g in range(5):
    _gs = GROUP_SIZES[_g]
    for _sm in range(2):
        for _j in range(_gs):
            _p = GROUP_P0[_g] + _sm * _gs + _j
            PART_SAMPLE[_p] = _sm
            PART_CHANNEL[_p] = 5 * _j + _g

# tiling: variable strips — a small first strip shortens the initial load
# the PE has to wait for; the last strip stores in finer blocks so the
# kernel tail is short.  Each strip entry is (h0, rows, chunk_plan,
# store_bounds): chunk_plan gives output rows per PSUM chunk (each
# N = rows*162 must be in [256, 512] so f32r streams at 1 col/cycle and
# fits one PSUM bank); store_bounds are chunk-aligned row offsets where
# store DMAs fire.
_STD_PLAN = [3, 3, 3, 3, 3, 3, 2] * 2
_LAST_PLAN = [3, 3, 3, 3, 3, 3, 2] + [2, 2, 3, 3] + [3, 3, 2, 2]
STRIP_LIST = [
    (0, 40, _STD_PLAN, (20, 40)),
    (40, 40, _STD_PLAN, (20, 40)),
    (80, 40, _STD_PLAN, (20, 40)),
    (120, 40, _LAST_PLAN, (20, 30, 36, 38, 40)),
]
MAX_STRIP = max(r for _, r, _, _ in STRIP_LIST)     # 48
MIN_STRIP = min(r for _, r, _, _ in STRIP_LIST)     # 24
XS_ROWS = MAX_STRIP + 3    # strip buffer rows (+1 halo each side +1 guard)

XS_BUFS = 3
STG_BUFS = 2
PSUM_BUFS = 4
WARMUP_MMS = 7             # dummy matmuls to ramp the PE during the head load
HEAD_CHUNKS = 6            # strip-0 chunks served by the small head tile
HEAD_ROWS = 22             # head tile rows (covers chunk HEAD_CHUNKS-1 + spill)
COPY_ENGINES = ("vector",)  # round-robin engines for PSUM->SBUF copies

TAPS = [(kh, kw) for kh in range(3) for kw in range(3)]


def build_kernel(reps: int = 1, timing: bool = False):
    nc = bacc.Bacc("TRN2", target_bir_lowering=False, debug=False,
                   num_devices=N_CORES)
    wt_dram = nc.dram_tensor("wt", [128, 9, 128], mybir.dt.float32r,
                             kind="ExternalInput")
    if timing:
        # timing-only variant: big tensors stay in device DRAM (uninitialised
        # garbage is fine for timing) so per-call host<->device transfer is
        # tiny and wall-clock noise is dominated by the fixed RTT only.
        x_dram = nc.dram_tensor("x", [2 * C, H, W], mybir.dt.float32r)
        out_dram = nc.dram_tensor("out", [B_PER_CORE, C, H, W],
                                  mybir.dt.float32)
        dummy = nc.dram_tensor("t_dummy", [1, 16], mybir.dt.float32r,
                               kind="ExternalOutput")
    else:
        x_dram = nc.dram_tensor("x", [2 * C, H, W], mybir.dt.float32r,
                                kind="ExternalInput")
        out_dram = nc.dram_tensor("out", [B_PER_CORE, C, H, W],
                                  mybir.dt.float32,
                                  kind="ExternalOutput")
    x_ap = x_dram.ap()
    out_flat = out_dram.ap().rearrange("b c h w -> (b c) h w")

    with tile.TileContext(nc) as tc:
        with (
            tc.tile_pool(name="wpool", bufs=1) as wpool,
            tc.tile_pool(name="xs_pool", bufs=XS_BUFS) as xs_pool,
            tc.tile_pool(name="stg_pool", bufs=STG_BUFS) as stg_pool,
            tc.tile_pool(name="psum", bufs=PSUM_BUFS, space="PSUM") as psum_pool,
        ):
            wt = wpool.tile([128, 9, 128], mybir.dt.float32r)
            nc.sync.dma_start(wt[:], wt_dram.ap()[:])

            # dummy matmuls over the weight tile while the first strip
            # loads: ramps the PE clock (HAM) so the real matmuls start at
            # full speed instead of paying the cold-clock penalty (they are
            # sized to end just after the first strip's loads land, keeping
            # the PE continuously busy into the real matmul stream)
            if WARMUP_MMS:
                wt_flat = wt[:].rearrange("p t m -> p (t m)")
                psw = psum_pool.tile([128, 512], mybir.dt.float32, tag="ps")
                for i in range(WARMUP_MMS):
                    nc.tensor.matmul(psw[:], wt[:, 0, :], wt_flat[:, 0:512],
                                     start=(i == 0),
                                     stop=(i == WARMUP_MMS - 1))

            # small head copy of the first rows of the image: loads fast
            # (~1/2 of a strip) so the first chunks' matmuls can start
            # ~8us before the full first strip has landed.  The full
            # strip-0 load below still covers these rows; the head is a
            # duplicate.
            xh = wpool.tile([128, HEAD_ROWS, VP], mybir.dt.float32r)
            nc.gpsimd.memset(xh[:].bitcast(mybir.dt.float32), 0.0)
            for g in range(5):
                p0, gs2 = GROUP_P0[g], 2 * GROUP_SIZES[g]
                dy, dx = DYS[g], DXS[g]
                u_lo = 1 + dy
                u_hi = min(HEAD_ROWS - 1, 160 + dy)
                eng = nc.sync if g % 2 == 0 else nc.scalar
                eng.dma_start(
                    xh[p0:p0 + gs2, u_lo:u_hi + 1, 1 + dx:161 + dx],
                    x_ap[p0:p0 + gs2, u_lo - 1 - dy:u_hi - dy, :],
                )
            xh_flat = xh[:].rearrange("p r v -> p (r v)")

            # one-time zeroing of the pad regions of each xs slot: the
            # left/right pad columns (never written by any load), and the
            # rows beyond what the smallest strip writes (guard rows for AP
            # spill + rows only larger strips own; loads rewrite the rest
            # every strip, and strips are sized so no slot ever shrinks in
            # a way that exposes stale rows within one rep).
            for _ in range(XS_BUFS):
                xi = xs_pool.tile([128, XS_ROWS, VP], mybir.dt.float32r,
                                  tag="xs")
                xif = xi[:].bitcast(mybir.dt.float32)
                nc.gpsimd.memset(xif[:, :, 0:2], 0.0)
                nc.gpsimd.memset(xif[:, :, VP - 2:VP], 0.0)
                nc.gpsimd.memset(xif[:, MIN_STRIP + 2:XS_ROWS, :], 0.0)

            for _ in range(reps):
                for s, (h0, srows, chunk_plan, out_bounds) in \
                        enumerate(STRIP_LIST):
                    xs = xs_pool.tile([128, XS_ROWS, VP], mybir.dt.float32r,
                                      tag="xs")
                    xsf = xs[:].bitcast(mybir.dt.float32)
                    # image-boundary rows the loads below leave unwritten
                    # (stale from the slot's previous strip): re-zero, the
                    # loads then overwrite whatever they do own.
                    if s == 0:
                        nc.gpsimd.memset(xsf[:, 0:2, :], 0.0)
                    if s == len(STRIP_LIST) - 1:
                        nc.gpsimd.memset(xsf[:, srows:srows + 2, :], 0.0)
                    # one load per shift group (both samples at once; the
                    # host packs both samples' group channels into one
                    # contiguous partition block); alternate the two HWDGE
                    # rings (SP / ACT) so descriptor prep runs in parallel
                    for g in range(5):
                        p0, gs2 = GROUP_P0[g], 2 * GROUP_SIZES[g]
                        dy, dx = DYS[g], DXS[g]
                        u_lo = max(h0, 1 + dy)
                        u_hi = min(h0 + srows + 1, 160 + dy)
                        eng = nc.sync if g % 2 == 0 else nc.scalar
                        eng.dma_start(
                            xs[p0:p0 + gs2,
                               u_lo - h0:u_hi - h0 + 1,
                               1 + dx:161 + dx],
                            x_ap[p0:p0 + gs2,
                                 u_lo - 1 - dy:u_hi - dy, :],
                        )
                    xs_flat = xs[:].rearrange("p r v -> p (r v)")
                    stg = stg_pool.tile([128, MAX_STRIP, W],
                                        mybir.dt.float32)
                    r0 = 0
                    for j, crows in enumerate(chunk_plan):
                        n_mm = VP * crows
                        ps = psum_pool.tile([128, 512], mybir.dt.float32,
                                            tag="ps")
                        src_flat = (xh_flat
                                    if s == 0 and j < HEAD_CHUNKS
                                    else xs_flat)
                        for t, (kh, kw) in enumerate(TAPS):
                            base = (r0 + kh) * VP + kw
                            nc.tensor.matmul(
                                ps[:, 0:n_mm],
                                wt[:, t, :],
                                src_flat[:, base:base + n_mm],
                                start=(t == 0),
                                stop=(t == len(TAPS) - 1),
                            )
                        ps_view = ps[:, 0:n_mm].rearrange(
                            "p (r v) -> p r v", v=VP)
                        ceng = getattr(
                            nc, COPY_ENGINES[j % len(COPY_ENGINES)])
                        ceng.tensor_copy(
                            stg[:, r0:r0 + crows, :],
                            ps_view[:, :, 0:W],
                        )
                        r0 += crows
                        # store each block as soon as its chunks are copied
                        # (SWDGE so stores can't head-of-line-block the
                        # HWDGE input loads).  One DMA covers both samples:
                        # the HBM (b c) dims are contiguous, matching the
                        # 128-partition staging layout.
                        if r0 in out_bounds:
                            rb = ([0] + [b for b in out_bounds if b < r0])[-1]
                            # last strip's stores go HWDGE (ACT): cheaper
                            # descriptor prep, and with no loads left there
                            # is nothing for them to head-of-line-block
                            seng = (nc.scalar if s == len(STRIP_LIST) - 1
                                    else nc.gpsimd)
                            seng.dma_start(
                                out_flat[:, h0 + rb:h0 + r0, :],
                                stg[:, rb:r0, :],
                            )
            if timing:
                nc.sync.dma_start(dummy.ap()[:], wt[0:1, 0, 0:16])
    nc.compile()
    return nc


def _round_fp32r(a: np.ndarray) -> np.ndarray:
    b = np.ascontiguousarray(a, dtype=np.float32).view(np.uint32)
    br = (b + np.uint32(0x800)) & np.uint32(0xFFFFF000)
    return br.view(np.float32)


def _host_inputs(x: np.ndarray, weight: np.ndarray):
    """Pack channels per shift group (both samples interleaved) and build
    the sample-block-masked stationary tap matrices."""
    xv = x.reshape(N_CORES, B_PER_CORE, C, H, W)
    # x_packed[k, p] = x[2k + sample(p), channel(p)]
    x_packed = _round_fp32r(xv[:, PART_SAMPLE, PART_CHANNEL])
    # lhsT[p, t, m] = weight[co(m), channel(p), t] iff sample(p)==sample(m)
    wk = weight.transpose(1, 2, 3, 0).reshape(C, 9, C)  # [ci, tap, co]
    wt_host = np.zeros((128, 9, 128), np.float32)
    for p in range(128):
        sm = PART_SAMPLE[p]
        wt_host[p, :, 64 * sm:64 * sm + 64] = wk[PART_CHANNEL[p]]
    wt_host = _round_fp32r(wt_host)
    return x_packed, wt_host


_NC_CACHE = {}


def _get_nc(reps: int = 1):
    if reps not in _NC_CACHE:
        _NC_CACHE[reps] = build_kernel(reps)
    return _NC_CACHE[reps]


def kernel(x: np.ndarray, weight: np.ndarray) -> np.ndarray:
    x = np.asarray(x, dtype=np.float32)
    weight = np.asarray(weight, dtype=np.float32)
    x_packed, wt_host = _host_inputs(x, weight)
    nc = _get_nc(1)
    in_maps = [
        {"x": np.ascontiguousarray(x_packed[k]), "wt": wt_host}
        for k in range(N_CORES)
    ]
    res = run_bass_kernel_spmd(nc, in_maps, core_ids=list(range(N_CORES)))
    out = np.empty((B, C, H, W), np.float32)
    for k in range(N_CORES):
        out[k * B_PER_CORE:(k + 1) * B_PER_CORE] = res.results[k]["out"]
    return out
